# revision 2
# baseline (speedup 1.0000x reference)
"""Trainium2 Bass kernel for masked multi-head attention (B=2, S=2048, H=16, D=64).

Sharding: 8 cores = (2 batches) x (4 groups of 4 heads). Each core computes
qkv for its 4 heads + flash-style attention fully on-chip.

Host-side prep per core:
  - xT    [1024, 2048] = x[b].T, pre-rounded to float32r (11-bit mantissa RNE)
  - w     [1024, 768]  = W_qkv columns for this head group (q|k|v), pre-rounded
  - bias  [128, n_kv/128] = 0 for real keys, -1e9 for padding (exp -> 0)
Device returns outT [260, 2048] = 4 heads x (64 out rows + 1 softmax-sum row),
host divides and transposes into the final [2, 2048, 1024].

Schedule: the attention inner loop is one flat software pipeline over
(block, j) steps; each step's exp (ACT engine) is the pacing resource, PE
fills gaps with QKV piece work, out-DMAs issue from the SP queue so the ACT
sequencer never stalls on them.
"""

import sys

sys.path.insert(0, "/opt/trn_rl_repo")

import numpy as np

import concourse.bass as bass  # noqa: F401
import concourse.tile as tile
from concourse import bacc, mybir
from concourse.bass_utils import run_bass_kernel_spmd

B, S, DIM = 2, 2048, 1024
HEAD, HEAD_DIM = 16, 64
NEG = np.float32(-1e9)
NFI = DIM // 128  # 8 contraction tiles
F32 = mybir.dt.float32
F32R = mybir.dt.float32r

N_DUMMY = 16

_CACHE = {}


def _round_f32r(x: np.ndarray) -> np.ndarray:
    """Round-to-nearest-even keeping 11 mantissa bits (hw float32r rounding)."""
    i = np.ascontiguousarray(x, dtype=np.float32).view(np.uint32).astype(np.uint64)
    shift = np.uint64(12)
    half = np.uint64(1 << 11)
    mask = np.uint64((1 << 12) - 1)
    r = (i + half - np.uint64(1) + ((i >> shift) & np.uint64(1))) & ~mask
    return r.astype(np.uint32).view(np.float32)


def _emit_body(nc, tc, pools, dram, n_kv):
    NKT = n_kv // 128
    big, ps_sc, ps_acc, ps_pv, ptp, osp = pools
    xT_d, w_d, bias_d, outT_d = dram
    Exp = mybir.ActivationFunctionType.Exp

    # preload the exp table while DMAs run
    warm = big.tile([128, 1], F32, tag="warm", name="warm")
    nc.gpsimd.memset(warm[:], 1.0)
    nc.scalar.activation(warm[:], warm[:], Exp)

    # dummy matmuls on a memset tile: keep PE busy through the DMA-gated
    # head so HAM reaches full clock before the real work arrives
    dummy = big.tile([128, 512], mybir.dt.bfloat16, tag="dummy", name="dummy")
    nc.gpsimd.memset(dummy[:], 1.0)
    dps = ps_acc.tile([128, 1024], F32, tag="acc", name="dps")
    for _ in range(N_DUMMY):
        nc.tensor.matmul(
            dps[:, 0:512], dummy[:, 0:128], dummy[:], start=True, stop=True
        )

    w_s = big.tile([128, NFI, 768], F32R, tag="w", name="w_s")
    w_ap = w_d.ap().rearrange("(a p) f -> p a f", p=128)
    bias_s = big.tile([128, NKT], F32, tag="bias", name="bias_s")
    xT_ap = xT_d.ap().rearrange("(a p) t -> p a t", p=128)
    xts = big.tile([128, NFI, S], F32R, tag="xts", name="xts")
    xkv = xts  # kept keys are the first n_kv (host-permuted) columns

    # DMA staging, strictly in first-need order (SP queue).
    # w cols: q_p0 0:128 | q_p1 128:256 | k_p0 256:384 | k_p1 384:512 | v 512:768
    nc.sync.dma_start(w_s[:, :, 256:384], w_ap[:, :, 256:384])  # w_k p0
    nc.sync.dma_start(bias_s[:], bias_d.ap())
    nc.sync.dma_start(xts[:, :, 0:256], xT_ap[:, :, 0:256])
    nc.sync.dma_start(xts[:, :, 256:512], xT_ap[:, :, 256:512])
    nc.sync.dma_start(w_s[:, :, 0:128], w_ap[:, :, 0:128])  # w_q p0
    nc.sync.dma_start(w_s[:, :, 512:768], w_ap[:, :, 512:768])  # w_v
    nc.sync.dma_start(xts[:, :, 512:768], xT_ap[:, :, 512:768])
    nc.sync.dma_start(xts[:, :, 768:1024], xT_ap[:, :, 768:1024])
    nc.sync.dma_start(w_s[:, :, 128:256], w_ap[:, :, 128:256])  # w_q p1
    nc.sync.dma_start(w_s[:, :, 384:512], w_ap[:, :, 384:512])  # w_k p1
    nc.sync.dma_start(xts[:, :, 1024:1536], xT_ap[:, :, 1024:1536])
    nc.sync.dma_start(xts[:, :, 1536:2048], xT_ap[:, :, 1536:2048])

    qT = [big.tile([128, S], F32R, tag=f"qT{p}", name=f"qT{p}") for p in range(2)]
    kT = [
        big.tile([128, n_kv], F32R, tag=f"kT{p}", name=f"kT{p}") for p in range(2)
    ]
    va = big.tile([128, NKT, 4, 65], F32R, tag="va", name="va")
    ones = big.tile([128, 4, 1], F32, tag="ones", name="ones")
    nc.gpsimd.memset(ones[:], 1.0)
    for jt in range(NKT):
        nc.vector.tensor_copy(va[:, jt, :, 64:65], ones[:])

    # ---- projection emitters (each is one ps_acc group: matmuls + copy) ----
    def emit_k_group(p, off, wd):
        acc = ps_acc.tile([128, 1024], F32, tag="acc", name="acc_k")
        for fi in range(NFI):
            nc.tensor.matmul(
                acc[:, :wd],
                w_s[:, fi, 256 + 128 * p : 256 + 128 * (p + 1)],
                xkv[:, fi, off : off + wd],
                start=(fi == 0),
                stop=(fi == NFI - 1),
            )
        nc.vector.tensor_copy(kT[p][:, off : off + wd], acc[:, :wd])

    def emit_v_group(jt):
        acc = ps_acc.tile([128, 1024], F32, tag="acc", name="acc_v")
        for fi in range(NFI):
            nc.tensor.matmul(
                acc[:, :256],
                xkv[:, fi, jt * 128 : (jt + 1) * 128],
                w_s[:, fi, 512:768],
                start=(fi == 0),
                stop=(fi == NFI - 1),
            )
        nc.vector.tensor_copy(
            va[:, jt, :, 0:64],
            acc[:, :256].rearrange("p (h d) -> p h d", h=4),
        )

    def emit_q_group(p, off, wd):
        acc = ps_acc.tile([128, 1024], F32, tag="acc", name="acc_q")
        for fi in range(NFI):
            nc.tensor.matmul(
                acc[:, :wd],
                w_s[:, fi, 128 * p : 128 * (p + 1)],
                xts[:, fi, off : off + wd],
                start=(fi == 0),
                stop=(fi == NFI - 1),
            )
        nc.vector.tensor_copy(qT[p][:, off : off + wd], acc[:, :wd])

    # ---- fine-grained filler queue: one matmul (or copy) per piece ----
    queue = []

    def q_pieces(p, off, wd, needed_by):
        cell = []

        def mk(fi):
            def f():
                if not cell:
                    cell.append(
                        ps_acc.tile([128, 1024], F32, tag="acc", name="acc_qf")
                    )
                nc.tensor.matmul(
                    cell[0][:, :wd],
                    w_s[:, fi, 128 * p : 128 * (p + 1)],
                    xts[:, fi, off : off + wd],
                    start=(fi == 0),
                    stop=(fi == NFI - 1),
                )

            return f

        for fi in range(NFI):
            queue.append((needed_by, mk(fi)))
        queue.append(
            (
                needed_by,
                lambda: nc.vector.tensor_copy(
                    qT[p][:, off : off + wd], cell[0][:, :wd]
                ),
            )
        )

    def k_pieces(p, off, wd, needed_by):
        cell = []

        def mk(fi):
            def f():
                if not cell:
                    cell.append(
                        ps_acc.tile([128, 1024], F32, tag="acc", name="acc_kf")
                    )
                nc.tensor.matmul(
                    cell[0][:, :wd],
                    w_s[:, fi, 256 + 128 * p : 256 + 128 * (p + 1)],
                    xkv[:, fi, off : off + wd],
                    start=(fi == 0),
                    stop=(fi == NFI - 1),
                )

            return f

        for fi in range(NFI):
            queue.append((needed_by, mk(fi)))
        queue.append(
            (
                needed_by,
                lambda: nc.vector.tensor_copy(
                    kT[p][:, off : off + wd], cell[0][:, :wd]
                ),
            )
        )

    def drain_required(bi):
        while queue and queue[0][0] <= bi:
            queue.pop(0)[1]()

    def pull(n):
        while n > 0 and queue:
            queue.pop(0)[1]()
            n -= 1

    # ---- upfront projections (DMA-arrival order) ----
    emit_k_group(0, 0, 256)
    emit_k_group(0, 256, 256)
    emit_q_group(0, 0, 512)

    # block-0 fillers keyed by j: k_p0 tail chunks before the scores that
    # need them, v groups right before their first consuming pv matmul
    inner_sc = {4: [lambda: emit_k_group(0, 512, 256)],
                6: [lambda: emit_k_group(0, 768, 256)]}
    inner_pv = {jt: [lambda jt=jt: emit_v_group(jt)] for jt in range(NKT)}

    # queue order mirrors DMA arrival: q_p0 c1-3 (x tail), then k_p1/q_p1
    for ci in range(1, 4):
        for half in range(2):
            q_pieces(0, 512 * ci + 256 * half, 256, needed_by=ci)
    for off in range(0, n_kv, 256):
        k_pieces(1, off, 256, needed_by=4)
    for ci in range(4):
        for half in range(2):
            q_pieces(1, 512 * ci + 256 * half, 256, needed_by=4 + ci)

    # ---- attention: flat software pipeline over (block, j) ----
    blocks = [(p, ci) for p in range(2) for ci in range(4)]

    def scores(p, ci, j):
        sc = ps_sc.tile([128, 1024], F32, tag="sc", name="sc")
        coff = 512 * ci
        for i in range(2):
            lo = 64 * i
            nc.tensor.matmul(
                sc[:, 512 * i : 512 * i + 512],
                kT[p][lo : lo + 64, j * 128 : (j + 1) * 128],
                qT[p][lo : lo + 64, coff : coff + 512],
                start=True,
                stop=True,
            )
        pt = ptp.tile([128, 1024], F32R, tag="pt", name="pt")
        nc.scalar.activation(pt[:], sc[:], Exp, bias=bias_s[:, j : j + 1])
        return pt

    def pv_mm(pv, p, j, pt):
        for i in range(2):
            nc.tensor.matmul(
                pv[:, 512 * i : 512 * i + 512],
                va[:, j, 2 * p + i, :],
                pt[:, 512 * i : 512 * i + 512],
                start=(j == 0),
                stop=(j == NKT - 1),
            )

    def finish(pv, p, ci):
        for i in range(2):
            o = osp.tile([65, 512], F32, tag="o", name="o")
            nc.vector.tensor_copy(o[:], pv[:, 512 * i : 512 * i + 512])
            lh = 2 * p + i
            nc.sync.dma_start(
                outT_d.ap()[65 * lh : 65 * lh + 65, 512 * ci : 512 * ci + 512],
                o[:],
            )

    pending = None  # (bi, p, ci, j, pt)
    pv_cur = None

    def emit_pending():
        nonlocal pending, pv_cur
        if pending is None:
            return
        pbi, pp, pci, pj, ppt = pending
        if pbi == 0:
            for f in inner_pv.get(pj, []):
                f()
        if pj == 0:
            pv_cur = ps_pv.tile([65, 1024], F32, tag="pv", name="pv")
        pv_mm(pv_cur, pp, pj, ppt)
        if pj == NKT - 1:
            finish(pv_cur, pp, pci)
        pending = None

    t = 0
    for bi, (p, ci) in enumerate(blocks):
        drain_required(bi)
        for j in range(NKT):
            if bi == 0:
                for f in inner_sc.get(j, []):
                    f()
            pt = scores(p, ci, j)
            if t >= 6:
                pull(3)
            emit_pending()
            pending = (bi, p, ci, j, pt)
            t += 1
    emit_pending()


def _build(n_kv: int, reps: int = 1):
    """Build the per-core Bass graph. Same graph runs SPMD on all 8 cores."""
    nc = bacc.Bacc("TRN2", target_bir_lowering=False, debug=False)

    NKT = n_kv // 128
    xT_d = nc.dram_tensor("xT", [DIM, S], F32R, kind="ExternalInput")
    w_d = nc.dram_tensor("w", [DIM, 768], F32R, kind="ExternalInput")
    bias_d = nc.dram_tensor("bias", [128, NKT], F32, kind="ExternalInput")
    outT_d = nc.dram_tensor("outT", [260, S], F32, kind="ExternalOutput")
    dram = (xT_d, w_d, bias_d, outT_d)

    with tile.TileContext(nc) as tc:
        with (
            tc.tile_pool(name="big", bufs=1) as big,
            tc.tile_pool(name="ps_sc", bufs=2, space="PSUM") as ps_sc,
            tc.tile_pool(name="ps_acc", bufs=1, space="PSUM") as ps_acc,
            tc.tile_pool(name="ps_pv", bufs=1, space="PSUM") as ps_pv,
            tc.tile_pool(name="ptp", bufs=6) as ptp,
            tc.tile_pool(name="osp", bufs=4) as osp,
        ):
            pools = (big, ps_sc, ps_acc, ps_pv, ptp, osp)
            for rep in range(reps):
                if rep:
                    tc.strict_bb_all_engine_barrier()
                _emit_body(nc, tc, pools, dram, n_kv)

    nc.compile()
    return nc


def _get_graph(n_kv: int, reps: int = 1):
    key = (n_kv, reps)
    if key not in _CACHE:
        _CACHE[key] = _build(n_kv, reps)
    return _CACHE[key]


def prepare(x, W_qkv, mask, reps: int = 1):
    """Host-side prep: returns (nc, in_maps, perms)."""
    x = np.asarray(x, dtype=np.float32)
    W_qkv = np.asarray(W_qkv, dtype=np.float32)
    mask = np.asarray(mask)

    keep = [np.nonzero(mask[b] != 0)[0] for b in range(B)]
    n_keep = max(len(k) for k in keep)
    n_kv = min(S, max(128, -(-n_keep // 128) * 128))

    # permute tokens: kept (unmasked) first, rest after; k/v use first n_kv
    perms, xT, biases = [], [], []
    for b in range(B):
        unkept = np.nonzero(mask[b] == 0)[0]
        perm = np.concatenate([keep[b], unkept])
        perms.append(perm)
        xT.append(_round_f32r(x[b][perm].T))
        bv = np.full(n_kv, NEG, np.float32)
        bv[: len(keep[b])] = 0.0
        biases.append(np.ascontiguousarray(bv.reshape(-1, 128).T))

    wg = []
    for g in range(4):
        cols = np.concatenate(
            [
                W_qkv[:, 256 * g : 256 * (g + 1)],
                W_qkv[:, 1024 + 256 * g : 1024 + 256 * (g + 1)],
                W_qkv[:, 2048 + 256 * g : 2048 + 256 * (g + 1)],
            ],
            axis=1,
        )
        wg.append(_round_f32r(cols))

    in_maps = []
    for c in range(8):
        b, g = c // 4, c % 4
        in_maps.append({"xT": xT[b], "w": wg[g], "bias": biases[b]})

    nc = _get_graph(n_kv, reps)
    return nc, in_maps, perms


def assemble(results, perms):
    out = np.empty((B, S, DIM), np.float32)
    for c in range(8):
        b, g = c // 4, c % 4
        outT = results[c]["outT"]
        for i in range(4):
            h = 4 * g + i
            rows = outT[65 * i : 65 * i + 64]
            sums = outT[65 * i + 64]
            out[b, perms[b], 64 * h : 64 * (h + 1)] = (rows / sums).T
    return out


def run(x, W_qkv, mask, trace=False, tmpdir=None):
    nc, in_maps, perms = prepare(x, W_qkv, mask)
    res = run_bass_kernel_spmd(
        nc, in_maps, core_ids=list(range(8)), trace=trace, tmpdir=tmpdir
    )
    return assemble(res.results, perms), res


def kernel(x, W_qkv, mask):
    out, _ = run(x, W_qkv, mask)
    return out


# revision 6
# speedup vs baseline: 1.0249x; 1.0249x over previous
"""Trainium2 Bass kernel for masked multi-head attention (B=2, S=2048, H=16, D=64).

Sharding: 8 cores = (2 batches) x (4 groups of 4 heads). Each core computes
qkv for its 4 heads + flash-style attention fully on-chip.

Host-side prep per core:
  - xT    [1024, 2048] = x[b].T, pre-rounded to float32r (11-bit mantissa RNE)
  - w     [1024, 768]  = W_qkv columns for this head group (q|k|v), pre-rounded
  - bias  [128, n_kv/128] = 0 for real keys, -1e9 for padding (exp -> 0)
Device returns outT [260, 2048] = 4 heads x (64 out rows + 1 softmax-sum row),
host divides and transposes into the final [2, 2048, 1024].

Schedule: the attention inner loop is one flat software pipeline over
(block, j) steps; each step's exp (ACT engine) is the pacing resource, PE
fills gaps with QKV piece work, out-DMAs issue from the SP queue so the ACT
sequencer never stalls on them.
"""

import sys

sys.path.insert(0, "/opt/trn_rl_repo")

import numpy as np

import concourse.bass as bass  # noqa: F401
import concourse.tile as tile
from concourse import bacc, mybir
from concourse.bass_utils import run_bass_kernel_spmd

B, S, DIM = 2, 2048, 1024
HEAD, HEAD_DIM = 16, 64
NEG = np.float32(-1e9)
NFI = DIM // 128  # 8 contraction tiles
F32 = mybir.dt.float32
F32R = mybir.dt.float32r

N_DUMMY = 20

_CACHE = {}


def _round_f32r(x: np.ndarray) -> np.ndarray:
    """Round-to-nearest-even keeping 11 mantissa bits (hw float32r rounding)."""
    i = np.ascontiguousarray(x, dtype=np.float32).view(np.uint32).astype(np.uint64)
    shift = np.uint64(12)
    half = np.uint64(1 << 11)
    mask = np.uint64((1 << 12) - 1)
    r = (i + half - np.uint64(1) + ((i >> shift) & np.uint64(1))) & ~mask
    return r.astype(np.uint32).view(np.float32)


def _emit_body(nc, tc, pools, dram, n_kv):
    NKT = n_kv // 128
    big, ps_sc, ps_acc, ps_pv, ptp, osp = pools
    xT_d, w_d, bias_d, outT_d = dram
    Exp = mybir.ActivationFunctionType.Exp

    # preload the exp table while DMAs run
    warm = big.tile([128, 1], F32, tag="warm", name="warm")
    nc.gpsimd.memset(warm[:], 1.0)
    nc.scalar.activation(warm[:], warm[:], Exp)

    # dummy matmuls on a memset tile: keep PE busy through the DMA-gated
    # head so HAM reaches full clock before the real work arrives
    dummy = big.tile([128, 512], mybir.dt.bfloat16, tag="dummy", name="dummy")
    nc.vector.memset(dummy[:], 1.0)
    dps = ps_acc.tile([128, 1024], F32, tag="acc", name="dps")
    for _ in range(N_DUMMY):
        nc.tensor.matmul(
            dps[:, 0:512], dummy[:, 0:128], dummy[:], start=True, stop=True
        )

    w_s = big.tile([128, NFI, 768], F32R, tag="w", name="w_s")
    w_ap = w_d.ap().rearrange("(a p) f -> p a f", p=128)
    bias_s = big.tile([128, NKT], F32, tag="bias", name="bias_s")
    xT_ap = xT_d.ap().rearrange("(a p) t -> p a t", p=128)
    xts = big.tile([128, NFI, S], F32R, tag="xts", name="xts")
    xkv = xts  # kept keys are the first n_kv (host-permuted) columns

    # DMA staging, strictly in first-need order (SP queue).
    # w cols: q_p0 0:128 | q_p1 128:256 | k_p0 256:384 | k_p1 384:512 | v 512:768
    nc.sync.dma_start(w_s[:, :, 256:384], w_ap[:, :, 256:384])  # w_k p0
    nc.sync.dma_start(bias_s[:], bias_d.ap())
    nc.sync.dma_start(xts[:, :, 0:256], xT_ap[:, :, 0:256])
    nc.sync.dma_start(xts[:, :, 256:512], xT_ap[:, :, 256:512])
    nc.sync.dma_start(w_s[:, :, 0:128], w_ap[:, :, 0:128])  # w_q p0
    nc.sync.dma_start(w_s[:, :, 512:768], w_ap[:, :, 512:768])  # w_v
    nc.sync.dma_start(xts[:, :, 512:768], xT_ap[:, :, 512:768])
    nc.sync.dma_start(xts[:, :, 768:1024], xT_ap[:, :, 768:1024])
    nc.sync.dma_start(w_s[:, :, 128:256], w_ap[:, :, 128:256])  # w_q p1
    nc.sync.dma_start(w_s[:, :, 384:512], w_ap[:, :, 384:512])  # w_k p1
    nc.sync.dma_start(xts[:, :, 1024:1536], xT_ap[:, :, 1024:1536])
    nc.sync.dma_start(xts[:, :, 1536:2048], xT_ap[:, :, 1536:2048])

    qT = [big.tile([128, S], F32R, tag=f"qT{p}", name=f"qT{p}") for p in range(2)]
    kT = [
        big.tile([128, n_kv], F32R, tag=f"kT{p}", name=f"kT{p}") for p in range(2)
    ]
    va = big.tile([128, NKT, 4, 65], F32R, tag="va", name="va")
    ones = big.tile([128, 4, 1], F32, tag="ones", name="ones")
    nc.gpsimd.memset(ones[:], 1.0)
    for jt in range(NKT):
        nc.vector.tensor_copy(va[:, jt, :, 64:65], ones[:])

    # ---- projection emitters (each is one ps_acc group: matmuls + copy) ----
    def emit_k_group(p, off, wd):
        acc = ps_acc.tile([128, 1024], F32, tag="acc", name="acc_k")
        for fi in range(NFI):
            nc.tensor.matmul(
                acc[:, :wd],
                w_s[:, fi, 256 + 128 * p : 256 + 128 * (p + 1)],
                xkv[:, fi, off : off + wd],
                start=(fi == 0),
                stop=(fi == NFI - 1),
            )
        nc.vector.tensor_copy(kT[p][:, off : off + wd], acc[:, :wd])

    def emit_v_group(jt):
        acc = ps_acc.tile([128, 1024], F32, tag="acc", name="acc_v")
        for fi in range(NFI):
            nc.tensor.matmul(
                acc[:, :256],
                xkv[:, fi, jt * 128 : (jt + 1) * 128],
                w_s[:, fi, 512:768],
                start=(fi == 0),
                stop=(fi == NFI - 1),
            )
        nc.vector.tensor_copy(
            va[:, jt, :, 0:64],
            acc[:, :256].rearrange("p (h d) -> p h d", h=4),
        )

    def emit_q_group(p, off, wd):
        acc = ps_acc.tile([128, 1024], F32, tag="acc", name="acc_q")
        for fi in range(NFI):
            nc.tensor.matmul(
                acc[:, :wd],
                w_s[:, fi, 128 * p : 128 * (p + 1)],
                xts[:, fi, off : off + wd],
                start=(fi == 0),
                stop=(fi == NFI - 1),
            )
        nc.vector.tensor_copy(qT[p][:, off : off + wd], acc[:, :wd])

    # ---- fine-grained filler queue: one matmul (or copy) per piece ----
    queue = []

    def q_pieces(p, off, wd, needed_by):
        cell = []

        def mk(fi):
            def f():
                if not cell:
                    cell.append(
                        ps_acc.tile([128, 1024], F32, tag="acc", name="acc_qf")
                    )
                nc.tensor.matmul(
                    cell[0][:, :wd],
                    w_s[:, fi, 128 * p : 128 * (p + 1)],
                    xts[:, fi, off : off + wd],
                    start=(fi == 0),
                    stop=(fi == NFI - 1),
                )

            return f

        for fi in range(NFI):
            queue.append((needed_by, mk(fi)))
        queue.append(
            (
                needed_by,
                lambda: nc.vector.tensor_copy(
                    qT[p][:, off : off + wd], cell[0][:, :wd]
                ),
            )
        )

    def k_pieces(p, off, wd, needed_by):
        cell = []

        def mk(fi):
            def f():
                if not cell:
                    cell.append(
                        ps_acc.tile([128, 1024], F32, tag="acc", name="acc_kf")
                    )
                nc.tensor.matmul(
                    cell[0][:, :wd],
                    w_s[:, fi, 256 + 128 * p : 256 + 128 * (p + 1)],
                    xkv[:, fi, off : off + wd],
                    start=(fi == 0),
                    stop=(fi == NFI - 1),
                )

            return f

        for fi in range(NFI):
            queue.append((needed_by, mk(fi)))
        queue.append(
            (
                needed_by,
                lambda: nc.vector.tensor_copy(
                    kT[p][:, off : off + wd], cell[0][:, :wd]
                ),
            )
        )

    def drain_required(bi):
        while queue and queue[0][0] <= bi:
            queue.pop(0)[1]()

    def pull(n):
        while n > 0 and queue:
            queue.pop(0)[1]()
            n -= 1

    # ---- upfront projections (DMA-arrival order) ----
    emit_k_group(0, 0, 256)
    emit_k_group(0, 256, 256)
    emit_q_group(0, 0, 512)

    # block-0 fillers keyed by j: k_p0 tail chunks before the scores that
    # need them, v groups right before their first consuming pv matmul
    inner_sc = {4: [lambda: emit_k_group(0, 512, 256)],
                6: [lambda: emit_k_group(0, 768, 256)]}
    inner_pv = {jt: [lambda jt=jt: emit_v_group(jt)] for jt in range(NKT)}

    # queue order mirrors DMA arrival: q_p0 c1-3 (x tail), then k_p1/q_p1
    for ci in range(1, 4):
        for half in range(2):
            q_pieces(0, 512 * ci + 256 * half, 256, needed_by=ci)
    for off in range(0, n_kv, 256):
        k_pieces(1, off, 256, needed_by=4)
    for ci in range(4):
        for half in range(2):
            q_pieces(1, 512 * ci + 256 * half, 256, needed_by=4 + ci)

    # ---- attention: flat software pipeline over (block, j) ----
    blocks = [(p, ci) for p in range(2) for ci in range(4)]

    def scores(p, ci, j):
        sc = ps_sc.tile([128, 1024], F32, tag="sc", name="sc")
        coff = 512 * ci
        for i in range(2):
            lo = 64 * i
            nc.tensor.matmul(
                sc[:, 512 * i : 512 * i + 512],
                kT[p][lo : lo + 64, j * 128 : (j + 1) * 128],
                qT[p][lo : lo + 64, coff : coff + 512],
                start=True,
                stop=True,
            )
        pt = ptp.tile([128, 1024], F32R, tag="pt", name="pt")
        nc.scalar.activation(pt[:], sc[:], Exp, bias=bias_s[:, j : j + 1])
        return pt

    def pv_mm(pv, p, j, pt):
        for i in range(2):
            nc.tensor.matmul(
                pv[:, 512 * i : 512 * i + 512],
                va[:, j, 2 * p + i, :],
                pt[:, 512 * i : 512 * i + 512],
                start=(j == 0),
                stop=(j == NKT - 1),
            )

    def finish(pv, p, ci):
        for i in range(2):
            o = osp.tile([65, 512], F32, tag="o", name="o")
            nc.vector.tensor_copy(o[:], pv[:, 512 * i : 512 * i + 512])
            lh = 2 * p + i
            nc.sync.dma_start(
                outT_d.ap()[65 * lh : 65 * lh + 65, 512 * ci : 512 * ci + 512],
                o[:],
            )

    pending = None  # (bi, p, ci, j, pt)
    pv_cur = None

    def emit_pending():
        nonlocal pending, pv_cur
        if pending is None:
            return
        pbi, pp, pci, pj, ppt = pending
        if pbi == 0:
            for f in inner_pv.get(pj, []):
                f()
        if pj == 0:
            pv_cur = ps_pv.tile([65, 1024], F32, tag="pv", name="pv")
        pv_mm(pv_cur, pp, pj, ppt)
        if pj == NKT - 1:
            finish(pv_cur, pp, pci)
        pending = None

    t = 0
    for bi, (p, ci) in enumerate(blocks):
        drain_required(bi)
        for j in range(NKT):
            if bi == 0:
                for f in inner_sc.get(j, []):
                    f()
            pt = scores(p, ci, j)
            if t >= 6:
                pull(3)
            emit_pending()
            pending = (bi, p, ci, j, pt)
            t += 1
    emit_pending()


def _build(n_kv: int, reps: int = 1):
    """Build the per-core Bass graph. Same graph runs SPMD on all 8 cores."""
    nc = bacc.Bacc("TRN2", target_bir_lowering=False, debug=False)

    NKT = n_kv // 128
    xT_d = nc.dram_tensor("xT", [DIM, S], F32R, kind="ExternalInput")
    w_d = nc.dram_tensor("w", [DIM, 768], F32R, kind="ExternalInput")
    bias_d = nc.dram_tensor("bias", [128, NKT], F32, kind="ExternalInput")
    outT_d = nc.dram_tensor("outT", [260, S], F32, kind="ExternalOutput")
    dram = (xT_d, w_d, bias_d, outT_d)

    with tile.TileContext(nc) as tc:
        with (
            tc.tile_pool(name="big", bufs=1) as big,
            tc.tile_pool(name="ps_sc", bufs=2, space="PSUM") as ps_sc,
            tc.tile_pool(name="ps_acc", bufs=1, space="PSUM") as ps_acc,
            tc.tile_pool(name="ps_pv", bufs=1, space="PSUM") as ps_pv,
            tc.tile_pool(name="ptp", bufs=6) as ptp,
            tc.tile_pool(name="osp", bufs=4) as osp,
        ):
            pools = (big, ps_sc, ps_acc, ps_pv, ptp, osp)
            for rep in range(reps):
                if rep:
                    tc.strict_bb_all_engine_barrier()
                _emit_body(nc, tc, pools, dram, n_kv)

    nc.compile()
    return nc


def _get_graph(n_kv: int, reps: int = 1):
    key = (n_kv, reps)
    if key not in _CACHE:
        _CACHE[key] = _build(n_kv, reps)
    return _CACHE[key]


def prepare(x, W_qkv, mask, reps: int = 1):
    """Host-side prep: returns (nc, in_maps, perms)."""
    x = np.asarray(x, dtype=np.float32)
    W_qkv = np.asarray(W_qkv, dtype=np.float32)
    mask = np.asarray(mask)

    keep = [np.nonzero(mask[b] != 0)[0] for b in range(B)]
    n_keep = max(len(k) for k in keep)
    n_kv = min(S, max(128, -(-n_keep // 128) * 128))

    # permute tokens: kept (unmasked) first, rest after; k/v use first n_kv
    perms, xT, biases = [], [], []
    for b in range(B):
        unkept = np.nonzero(mask[b] == 0)[0]
        perm = np.concatenate([keep[b], unkept])
        perms.append(perm)
        xT.append(_round_f32r(x[b][perm].T))
        bv = np.full(n_kv, NEG, np.float32)
        bv[: len(keep[b])] = 0.0
        biases.append(np.ascontiguousarray(bv.reshape(-1, 128).T))

    wg = []
    for g in range(4):
        cols = np.concatenate(
            [
                W_qkv[:, 256 * g : 256 * (g + 1)],
                W_qkv[:, 1024 + 256 * g : 1024 + 256 * (g + 1)],
                W_qkv[:, 2048 + 256 * g : 2048 + 256 * (g + 1)],
            ],
            axis=1,
        )
        wg.append(_round_f32r(cols))

    in_maps = []
    for c in range(8):
        b, g = c // 4, c % 4
        in_maps.append({"xT": xT[b], "w": wg[g], "bias": biases[b]})

    nc = _get_graph(n_kv, reps)
    return nc, in_maps, perms


def assemble(results, perms):
    out = np.empty((B, S, DIM), np.float32)
    for c in range(8):
        b, g = c // 4, c % 4
        outT = results[c]["outT"]
        for i in range(4):
            h = 4 * g + i
            rows = outT[65 * i : 65 * i + 64]
            sums = outT[65 * i + 64]
            out[b, perms[b], 64 * h : 64 * (h + 1)] = (rows / sums).T
    return out


def run(x, W_qkv, mask, trace=False, tmpdir=None):
    nc, in_maps, perms = prepare(x, W_qkv, mask)
    res = run_bass_kernel_spmd(
        nc, in_maps, core_ids=list(range(8)), trace=trace, tmpdir=tmpdir
    )
    return assemble(res.results, perms), res


def kernel(x, W_qkv, mask):
    out, _ = run(x, W_qkv, mask)
    return out
